# revision 5
# baseline (speedup 1.0000x reference)
"""MultiOutSizeLinear (MoE-style routed linear) for Trainium2, 8 NeuronCores.

Each token selects one of 4 experts by its ``out_feat_size`` value
(128/256/512/1024). Expert k is a dense [out_k, 1024] linear + bias whose
output lands in the first out_k columns of the 1024-wide output row; the
reference leaves bias[k, out_k:] in the remaining columns (zero for the
shipped setup_inputs, which pre-zeroes the bias tail).

Strategy
  host:   route tokens to experts; balance each expert's tokens evenly
          across the 8 cores (capacities are shared so one SPMD program
          serves all cores); gather + transpose each core's tokens into
          x^T [1024, TPAD] laid out as expert segments [e3 | e2 | e1 | e0].
  device: keep W^T [1024, 1920] (all experts, concatenated out-columns) and
          a 128-row broadcast bias resident in SBUF. Stream 512-token
          chunks of x^T over the ACT HWDGE ring. All tensors that feed the
          PE are declared float32r (raw fp32 bits; the PE's full-rate fp32
          mode, ~1.3e-4 relative error vs fp32). Experts 1-3 run
          token-stationary: psum[128 tok, out_k] += xT_tile.T @ wT_tile,
          8 accumulating K-tiles per <=512-wide column chunk. Expert 0
          (out=128, too narrow for full-rate f32r) runs weight-stationary:
          psum[128 out, 512 tok] = out0^T chunks. Bias is added on VectorE
          during PSUM eviction (expert 0's bias is added on the host).
          Compact per-expert outputs go back over the SP HWDGE ring.
  host:   scatter rows back through the routing permutation.
"""

import sys
import numpy as np

sys.path.insert(0, "/opt/trn_rl_repo")

OUT_SIZES = (128, 256, 512, 1024)
N_EXPERTS = len(OUT_SIZES)
IN_FEAT = 1024
N_CORES = 8
K_TILES = IN_FEAT // 128
CHUNK = 512  # tokens per x^T DMA
WOFF = tuple(int(np.cumsum((0,) + OUT_SIZES)[k]) for k in range(N_EXPERTS))
W_COLS = sum(OUT_SIZES)

_nc_cache: dict = {}


def _build(caps, repeat=1, loop=None, xbufs=6, obufs=4):
    """Compile the SPMD program for shared per-expert capacities ``caps``.

    caps[0] % 512 == 0, caps[1]+caps[2]+caps[3] % 512 == 0, each % 128 == 0.
    ``repeat``/``loop`` re-run the compute body (same I/O) for timing.
    """
    import concourse.bacc as bacc
    import concourse.mybir as mybir
    import concourse.tile as tile

    f32 = mybir.dt.float32
    f32r = mybir.dt.float32r
    tpad = sum(caps)
    assert tpad % CHUNK == 0 and caps[0] % 512 == 0
    assert (caps[1] + caps[2] + caps[3]) % 512 == 0

    nc = bacc.Bacc(None, target_bir_lowering=False, debug=False)
    # chunk-blocked x^T: block c holds tokens [c*CHUNK, (c+1)*CHUNK) as a
    # contiguous [IN_FEAT, CHUNK] slab -> each chunk DMA is one fully
    # sequential 2 MB HBM read (strided reads measured ~1.8x slower)
    xt = nc.dram_tensor("xt", [tpad // CHUNK, IN_FEAT, CHUNK], f32r,
                        kind="ExternalInput")
    wt = nc.dram_tensor("wt", [IN_FEAT, W_COLS], f32r, kind="ExternalInput")
    bb = nc.dram_tensor("bb", [128, W_COLS], f32, kind="ExternalInput")
    outs = {}
    for k in (1, 2, 3):
        if caps[k]:
            outs[k] = nc.dram_tensor(f"out{k}", [caps[k], OUT_SIZES[k]], f32,
                                     kind="ExternalOutput")
    if caps[0]:
        outs[0] = nc.dram_tensor("out0t", [128, caps[0]], f32,
                                 kind="ExternalOutput")

    seg_order = [k for k in (3, 2, 1, 0) if caps[k] > 0]
    seg_start = {}
    t0 = 0
    for k in seg_order:
        seg_start[k] = t0
        t0 += caps[k]

    def expert_of(tok):
        for k in seg_order:
            if tok < seg_start[k] + caps[k]:
                return k
        raise AssertionError


    with tile.TileContext(nc) as tc:
        with (
            tc.tile_pool(name="const", bufs=1) as const,
            tc.tile_pool(name="xp", bufs=xbufs) as xp,
            tc.tile_pool(name="op", bufs=obufs) as op,
            tc.tile_pool(name="ps", bufs=3, space="PSUM") as psp,
            tc.tile_pool(name="ps0", bufs=2, space="PSUM") as psp0,
        ):
            wt_sb = const.tile([128, K_TILES, W_COLS], f32r)
            nc.sync.dma_start(wt_sb[:], wt.rearrange("(kk p) n -> p kk n", p=128))
            bb_sb = const.tile([128, W_COLS], f32)
            nc.sync.dma_start(bb_sb[:], bb[:])

            def body():
                for c0 in range(0, tpad, CHUNK):
                    x_sb = xp.tile([128, K_TILES, CHUNK], f32r, tag="x")
                    nc.scalar.dma_start(
                        x_sb[:],
                        xt[c0 // CHUNK].rearrange("(kk p) t -> p kk t", p=128))
                    g0 = 0
                    while g0 < CHUNK:
                        tok = c0 + g0
                        k = expert_of(tok)
                        if k == 0:
                            # weight-stationary: psum = out0^T [128 out, 512 tok]
                            ps = psp0.tile([128, 512], f32, tag="ps0")
                            for kk in range(K_TILES):
                                nc.tensor.matmul(
                                    ps[:],
                                    wt_sb[:, kk, WOFF[0]:WOFF[0] + 128],
                                    x_sb[:, kk, g0:g0 + 512],
                                    start=(kk == 0), stop=(kk == K_TILES - 1))
                            o_sb = op.tile([128, 512], f32, tag="o0")
                            nc.vector.tensor_copy(o_sb[:], ps[:])
                            row = tok - seg_start[0]
                            nc.sync.dma_start(outs[0][:, row:row + 512], o_sb[:])
                            g0 += 512
                            continue
                        ok = OUT_SIZES[k]
                        ps = psp.tile([128, 1024], f32, tag="ps")
                        for j0 in range(0, ok, 512):
                            jn = min(512, ok - j0)
                            for kk in range(K_TILES):
                                nc.tensor.matmul(
                                    ps[:, j0:j0 + jn],
                                    x_sb[:, kk, g0:g0 + 128],
                                    wt_sb[:, kk, WOFF[k] + j0:WOFF[k] + j0 + jn],
                                    start=(kk == 0), stop=(kk == K_TILES - 1))
                        o_sb = op.tile([128, 1024], f32, tag="o")
                        nc.vector.tensor_add(o_sb[:, :ok], ps[:, :ok],
                                             bb_sb[:, WOFF[k]:WOFF[k] + ok])
                        row = tok - seg_start[k]
                        nc.sync.dma_start(outs[k][row:row + 128, :], o_sb[:, :ok])
                        g0 += 128

            if loop:
                with tc.For_i(0, loop, 1):
                    body()
            else:
                for _ in range(repeat):
                    body()
    nc.compile()
    return nc


def _get_nc(caps, repeat=1, loop=None):
    key = (tuple(caps), repeat, loop)
    if key not in _nc_cache:
        _nc_cache[key] = _build(caps, repeat=repeat, loop=loop)
    return _nc_cache[key]


def _route(out_feat_size):
    """Map out_feat_size values -> expert index (-1 = matches no expert)."""
    ofs = np.asarray(out_feat_size).astype(np.int64).reshape(-1)
    branch = np.full(ofs.shape, -1, dtype=np.int64)
    for k, s in enumerate(OUT_SIZES):
        branch[ofs == s] = k
    return branch


def _plan(branch):
    """Balanced routing plan: per-expert global index lists split evenly
    across cores, shared capacities, and segment layout [3,2,1,0]."""
    idx_all = {k: np.nonzero(branch == k)[0] for k in range(N_EXPERTS)}
    per_core = [int(-(-len(idx_all[k]) // N_CORES)) for k in range(N_EXPERTS)]
    caps = [int(-(-per_core[k] // 128) * 128) for k in range(N_EXPERTS)]
    # alignment: caps0 % 512, (caps1+2+3) % 512
    if caps[0] % 512:
        caps[0] += 512 - caps[0] % 512
    rem = (caps[1] + caps[2] + caps[3]) % 512
    if rem:
        for k in (1, 2, 3):  # pad the cheapest non-empty of e1..e3
            if caps[k]:
                caps[k] += 512 - rem
                break
        else:
            caps[0] += (512 - rem) if caps[0] else 0
    return idx_all, tuple(caps)


def kernel(x, weight, bias, out_feat_size):
    from concourse.bass_utils import run_bass_kernel_spmd

    x = np.asarray(x, dtype=np.float32)
    weight = np.asarray(weight, dtype=np.float32)
    bias = np.asarray(bias, dtype=np.float32)
    B, T, D = x.shape
    assert D == IN_FEAT
    n_tok = B * T

    branch = _route(out_feat_size)
    idx_all, caps = _plan(branch)
    if sum(caps) == 0:
        return np.zeros((B, T, IN_FEAT), dtype=np.float32)

    # host-side weight/bias layout
    wt = np.empty((IN_FEAT, W_COLS), dtype=np.float32)
    bb = np.empty((W_COLS,), dtype=np.float32)
    for k, ok in enumerate(OUT_SIZES):
        wt[:, WOFF[k]:WOFF[k] + ok] = weight[k, :ok, :].T
        bb[WOFF[k]:WOFF[k] + ok] = bias[k, :ok]
    bb128 = np.ascontiguousarray(np.broadcast_to(bb, (128, W_COLS)))

    x2 = x.reshape(n_tok, IN_FEAT)
    tpad = sum(caps)
    seg_off = {}
    t0 = 0
    for k in (3, 2, 1, 0):
        if caps[k]:
            seg_off[k] = t0
            t0 += caps[k]

    in_maps = []
    core_slices = []  # per core: {expert: global idx array}
    for c in range(N_CORES):
        perm = np.zeros(tpad, dtype=np.int64)
        slices = {}
        for k, off in seg_off.items():
            idx = idx_all[k]
            m = int(-(-len(idx) // N_CORES))
            part = idx[c * m:(c + 1) * m]
            slices[k] = part
            if len(part):
                perm[off:off + len(part)] = part
                perm[off + len(part):off + caps[k]] = part[0]
        xtb = np.empty((tpad // CHUNK, IN_FEAT, CHUNK), dtype=np.float32)
        for ci in range(tpad // CHUNK):
            np.copyto(xtb[ci], x2[perm[ci * CHUNK:(ci + 1) * CHUNK]].T)
        in_maps.append({"xt": xtb, "wt": wt, "bb": bb128})
        core_slices.append(slices)

    global _LAST_CAPS, _LAST_IN_MAPS
    _LAST_CAPS, _LAST_IN_MAPS = caps, in_maps

    nc = _get_nc(caps)
    res = run_bass_kernel_spmd(nc, in_maps, list(range(N_CORES))).results

    out = np.zeros((n_tok, IN_FEAT), dtype=np.float32)
    for c in range(N_CORES):
        for k, part in core_slices[c].items():
            n = len(part)
            if n == 0:
                continue
            ok = OUT_SIZES[k]
            if k == 0:
                out[part, :ok] = res[c]["out0t"][:, :n].T + bias[0, :ok]
            else:
                out[part, :ok] = res[c][f"out{k}"][:n]
            if ok < IN_FEAT:
                # reference semantics: bias tail beyond out_k (zero for the
                # shipped inputs, which pre-zero the bias)
                out[part, ok:] = bias[k, ok:]
    return out.reshape(B, T, IN_FEAT)
